# revision 5
# baseline (speedup 1.0000x reference)
"""ALiBi bias subtraction on 8 TRN2 NeuronCores.

out[b,h,q,k] = scores[b,h,q,k] - slopes[h] * (pos[q] - pos[k])

Sharding: head-parallel, 2 heads per core (16 heads / 8 cores), full batch
on every core. Per core: 128 MiB in + 128 MiB out of scores traffic.

Device kernel: for each (head, row-tile) the 4 batches are laid side by
side in the free dim -> one [128, 8192] f32 tile (4 MiB DMA).  The whole
ALiBi op is ONE DVE instruction per tile:
    out = (scores - rowbias[p]) + colbias[j]
with rowbias a per-partition scalar (slope*pos[q]) and colbias a
broadcast SBUF tile (slope*pos[k], constant per head).
"""

import numpy as np

import concourse.bass as bass  # noqa: F401  (AP types)
import concourse.mybir as mybir
from concourse import bacc
from concourse.tile import TileContext
from concourse.bass_utils import run_bass_kernel_spmd

B, H, S = 4, 16, 2048
NCORES = 8
HPC = H // NCORES  # heads per core = 2
P = 128            # partitions
RT = S // P        # row tiles per slice = 16
W = B * S          # free width with batches folded = 8192
WORK_BUFS = 3

_F32 = mybir.dt.float32

_cached_nc = None


def _build_nc():
    global _cached_nc
    if _cached_nc is not None:
        return _cached_nc

    nc = bacc.Bacc(
        "TRN2",
        target_bir_lowering=False,
        debug=False,
        num_devices=NCORES,
    )

    sc = nc.declare_dram_parameter("scores", [B, HPC, S, S], _F32, isOutput=False)
    cb_d = nc.declare_dram_parameter("cbias", [HPC, P, S], _F32, isOutput=False)
    rb_d = nc.declare_dram_parameter("rbias", [HPC, P, RT], _F32, isOutput=False)
    out_d = nc.declare_dram_parameter("out", [B, HPC, S, S], _F32, isOutput=True)

    with TileContext(nc) as tc:
        with (
            tc.tile_pool(name="const", bufs=1) as cpool,
            tc.tile_pool(name="cbpool", bufs=2) as cbpool,
            tc.tile_pool(name="work", bufs=3) as wpool,
        ):
            # per-partition row-bias scalars: rb[p, hi*RT + r] = slope*pos[r*128+p]
            rb = cpool.tile([P, HPC * RT], _F32)
            nc.sync.dma_start(
                out=rb.rearrange("p (h r) -> p h r", h=HPC),
                in_=rb_d[:].rearrange("h p r -> p h r"),
            )
            for hi in range(HPC):
                # col-bias tile, broadcast across partitions, constant per head
                cb = cbpool.tile([P, S], _F32, tag="cb")
                nc.sync.dma_start(out=cb[:], in_=cb_d[hi])
                # stride-0 view [P, B, S]: reads the same row per batch
                cb_bc = cb.unsqueeze(1).broadcast_to([P, B, S])
                for r in range(RT):
                    t = wpool.tile([P, W], _F32, tag="t")
                    t3 = t.rearrange("p (b j) -> p b j", b=B)
                    src = sc[:, hi, r * P : (r + 1) * P, :].rearrange("b p j -> p b j")
                    nc.sync.dma_start(out=t3, in_=src)
                    nc.vector.scalar_tensor_tensor(
                        out=t3,
                        in0=t3,
                        scalar=rb[:, hi * RT + r : hi * RT + r + 1],
                        in1=cb_bc,
                        op0=mybir.AluOpType.subtract,
                        op1=mybir.AluOpType.add,
                    )
                    dst = out_d[:, hi, r * P : (r + 1) * P, :].rearrange(
                        "b p j -> p b j"
                    )
                    # store on the ACT HWDGE ring so loads (SP ring) and
                    # stores run on separate DGE queues
                    nc.scalar.dma_start(out=dst, in_=t3)

    nc.compile()
    _cached_nc = nc
    return nc


def _host_prep(scores, slopes, positions, offset):
    scores = np.asarray(scores, dtype=np.float32)
    slopes = np.asarray(slopes, dtype=np.float32)
    positions = np.asarray(positions, dtype=np.float32)
    off = np.float32(np.asarray(offset))

    pos = positions[:S] + off                      # [S]
    sp = slopes[:, None] * pos[None, :]            # [H, S]  slope*pos

    in_maps = []
    for c in range(NCORES):
        h0 = c * HPC
        shard = np.ascontiguousarray(scores[:, h0 : h0 + HPC])  # [B, HPC, S, S]
        cb = np.empty((HPC, P, S), dtype=np.float32)
        rb = np.empty((HPC, P, RT), dtype=np.float32)
        for hi in range(HPC):
            cb[hi] = sp[h0 + hi][None, :]          # colbias, broadcast over partitions
            rb[hi] = sp[h0 + hi].reshape(RT, P).T  # [P, RT]
        in_maps.append({"scores": shard, "cbias": cb, "rbias": rb})
    return in_maps


def run(scores, slopes, positions, offset, trace=False):
    nc = _build_nc()
    in_maps = _host_prep(scores, slopes, positions, offset)
    res = run_bass_kernel_spmd(
        nc, in_maps, core_ids=list(range(NCORES)), trace=trace
    )
    full = np.empty((B, H, S, S), dtype=np.float32)
    for c in range(NCORES):
        full[:, c * HPC : (c + 1) * HPC] = res.results[c]["out"]
    return full, res


def kernel(scores, slopes, positions, offset):
    out, _ = run(scores, slopes, positions, offset, trace=False)
    return out


# revision 9
# speedup vs baseline: 1.1666x; 1.1666x over previous
"""ALiBi bias subtraction on 8 TRN2 NeuronCores.

out[b,h,q,k] = scores[b,h,q,k] - slopes[h] * (pos[q] - pos[k])

Sharding: head-parallel, 2 heads per core (16 heads / 8 cores), full batch
on every core. Per core: 128 MiB in + 128 MiB out of scores traffic.

Device kernel: for each (head, row-tile) the 4 batches are laid side by
side in the free dim -> one [128, 8192] f32 tile (4 MiB DMA).  The whole
ALiBi op is ONE DVE instruction per tile:
    out = (scores - rowbias[p]) + colbias[j]
with rowbias a per-partition scalar (slope*pos[q]) and colbias a
broadcast SBUF tile (slope*pos[k], constant per head).
"""

import numpy as np

import concourse.bass as bass  # noqa: F401  (AP types)
import concourse.mybir as mybir
from concourse import bacc
from concourse.tile import TileContext
from concourse.bass_utils import run_bass_kernel_spmd

import os

B, H, S = 4, 16, 2048
NCORES = 8
HPC = H // NCORES  # heads per core = 2
P = 128            # partitions
RT = S // P        # row tiles per slice = 16
W = B * S          # free width with batches folded = 8192
WORK_BUFS = int(os.environ.get("K_WORK_BUFS", "3"))
CB_BCAST = os.environ.get("K_CB_BCAST", "1") == "1"

_F32 = mybir.dt.float32

_cached_nc = None


def _build_nc():
    global _cached_nc
    if _cached_nc is not None:
        return _cached_nc

    nc = bacc.Bacc(
        "TRN2",
        target_bir_lowering=False,
        debug=False,
        num_devices=NCORES,
    )

    cb_w = S if CB_BCAST else W
    sc = nc.declare_dram_parameter("scores", [B, HPC, S, S], _F32, isOutput=False)
    cb_d = nc.declare_dram_parameter("cbias", [HPC, P, cb_w], _F32, isOutput=False)
    rb_d = nc.declare_dram_parameter("rbias", [HPC, P, RT], _F32, isOutput=False)
    out_d = nc.declare_dram_parameter("out", [B, HPC, S, S], _F32, isOutput=True)

    with TileContext(nc) as tc:
        with (
            tc.tile_pool(name="const", bufs=1) as cpool,
            tc.tile_pool(name="cbpool", bufs=2) as cbpool,
            tc.tile_pool(name="work", bufs=3) as wpool,
        ):
            # per-partition row-bias scalars: rb[p, hi*RT + r] = slope*pos[r*128+p]
            rb = cpool.tile([P, HPC * RT], _F32)
            nc.sync.dma_start(
                out=rb.rearrange("p (h r) -> p h r", h=HPC),
                in_=rb_d[:].rearrange("h p r -> p h r"),
            )
            for hi in range(HPC):
                # col-bias tile, broadcast across partitions, constant per head
                cb = cbpool.tile([P, cb_w], _F32, tag="cb")
                nc.sync.dma_start(out=cb[:], in_=cb_d[hi])
                if CB_BCAST:
                    # stride-0 view [P, B, S]: reads the same row per batch
                    cb_bc = cb.unsqueeze(1).broadcast_to([P, B, S])
                else:
                    cb_bc = cb.rearrange("p (b j) -> p b j", b=B)
                for r in range(RT):
                    t = wpool.tile([P, W], _F32, tag="t")
                    t3 = t.rearrange("p (b j) -> p b j", b=B)
                    src = sc[:, hi, r * P : (r + 1) * P, :].rearrange("b p j -> p b j")
                    nc.sync.dma_start(out=t3, in_=src)
                    nc.vector.scalar_tensor_tensor(
                        out=t3,
                        in0=t3,
                        scalar=rb[:, hi * RT + r : hi * RT + r + 1],
                        in1=cb_bc,
                        op0=mybir.AluOpType.subtract,
                        op1=mybir.AluOpType.add,
                    )
                    dst = out_d[:, hi, r * P : (r + 1) * P, :].rearrange(
                        "b p j -> p b j"
                    )
                    # store on the ACT HWDGE ring so loads (SP ring) and
                    # stores run on separate DGE queues
                    nc.scalar.dma_start(out=dst, in_=t3)

    nc.compile()
    _cached_nc = nc
    return nc


def _host_prep(scores, slopes, positions, offset):
    scores = np.asarray(scores, dtype=np.float32)
    slopes = np.asarray(slopes, dtype=np.float32)
    positions = np.asarray(positions, dtype=np.float32)
    off = np.float32(np.asarray(offset))

    pos = positions[:S] + off                      # [S]
    sp = slopes[:, None] * pos[None, :]            # [H, S]  slope*pos

    in_maps = []
    for c in range(NCORES):
        h0 = c * HPC
        shard = np.ascontiguousarray(scores[:, h0 : h0 + HPC])  # [B, HPC, S, S]
        cb_w = S if CB_BCAST else W
        cb = np.empty((HPC, P, cb_w), dtype=np.float32)
        rb = np.empty((HPC, P, RT), dtype=np.float32)
        for hi in range(HPC):
            row = sp[h0 + hi] if CB_BCAST else np.tile(sp[h0 + hi], B)
            cb[hi] = row[None, :]                  # colbias, broadcast over partitions
            rb[hi] = sp[h0 + hi].reshape(RT, P).T  # [P, RT]
        in_maps.append({"scores": shard, "cbias": cb, "rbias": rb})
    return in_maps


def run(scores, slopes, positions, offset, trace=False):
    nc = _build_nc()
    in_maps = _host_prep(scores, slopes, positions, offset)
    res = run_bass_kernel_spmd(
        nc, in_maps, core_ids=list(range(NCORES)), trace=trace
    )
    full = np.empty((B, H, S, S), dtype=np.float32)
    for c in range(NCORES):
        full[:, c * HPC : (c + 1) * HPC] = res.results[c]["out"]
    return full, res


def kernel(scores, slopes, positions, offset):
    out, _ = run(scores, slopes, positions, offset, trace=False)
    return out
